# revision 1
# baseline (speedup 1.0000x reference)
"""Hard triplet loss over SoftDTW self-distances — TRN2 Bass kernel.

Sharding: data-parallel over the B=128 signatures, 16 per core on 8 cores.
Device stage computes per-row squared norms of each signature shard
(the memory-heavy streaming pass over the full 8MB input); host combines
with the Gram matrices and runs the sequential SoftDTW anti-diagonal DP
and the tiny triplet epilogue in float32.
"""
import numpy as np

import concourse.bass as bass
import concourse.mybir as mybir
from concourse.bass_utils import run_bass_kernel_spmd

NG, NF, NW = 5, 10, 8
STEP = NG + NF + 1          # 16
MARGIN = np.float32(1.0)
MODEL_LAMBDA = np.float32(0.01)
GAMMA = np.float32(5.0)
BIG = np.float32(1e9)

B, N, F = 128, 512, 32
NCORES = 8
SIGS = B // NCORES          # 16 signatures per core
TILES = SIGS * (N // 128)   # 64 [128,32] tiles per core


def _build_norms_kernel():
    nc = bass.Bass()
    x = nc.declare_dram_parameter("x", [SIGS, N, F], mybir.dt.float32, isOutput=False)
    out = nc.declare_dram_parameter("norms", [128, TILES], mybir.dt.float32, isOutput=True)

    with (
        nc.sbuf_tensor([128, 2 * F], mybir.dt.float32) as in_tile,
        nc.sbuf_tensor([128, F], mybir.dt.float32) as sq_tile,
        nc.sbuf_tensor([128, TILES], mybir.dt.float32) as out_sbuf,
        nc.semaphore("dsem") as dsem,
        nc.semaphore("vsem") as vsem,
        nc.Block() as block,
    ):
        @block.sync
        def _(sync: bass.BassEngine):
            for i in range(TILES):
                s, q = divmod(i, N // 128)
                b = i % 2
                if i >= 2:
                    sync.wait_ge(vsem, i - 1)
                sync.dma_start(
                    out=in_tile[:, b * F:(b + 1) * F],
                    in_=x[s, q * 128:(q + 1) * 128, :],
                ).then_inc(dsem, 16)
            sync.wait_ge(vsem, TILES)
            sync.dma_start(out=out[:, :], in_=out_sbuf[:, :]).then_inc(dsem, 16)
            sync.wait_ge(dsem, 16 * (TILES + 1))

        @block.vector
        def _(vector: bass.BassEngine):
            for i in range(TILES):
                b = i % 2
                vector.wait_ge(dsem, 16 * (i + 1))
                src = in_tile[:, b * F:(b + 1) * F]
                vector.tensor_mul(sq_tile[:, :], src, src)
                vector.reduce_sum(
                    out_sbuf[:, i:i + 1], sq_tile[:, :], axis=mybir.AxisListType.X
                ).then_inc(vsem, 1)

    return nc


def _softmin3(a, b, c):
    s = np.float32(-1.0 / GAMMA)
    xa, xb, xc = a * s, b * s, c * s
    m = np.maximum(np.maximum(xa, xb), xc)
    lse = m + np.log(np.exp(xa - m) + np.exp(xb - m) + np.exp(xc - m))
    return -GAMMA * lse


def _shift(v):
    return np.concatenate([np.full((v.shape[0], 1), BIG, np.float32), v[:, :-1]], axis=1)


def kernel(data: np.ndarray, lens: np.ndarray) -> np.ndarray:
    data = np.asarray(data, np.float32)
    lens = np.asarray(lens, np.int32)

    nc = _build_norms_kernel()
    in_maps = [{"x": np.ascontiguousarray(data[c * SIGS:(c + 1) * SIGS])}
               for c in range(NCORES)]
    res = run_bass_kernel_spmd(nc, in_maps, list(range(NCORES)))

    # norms[p, s*4+q] = sum_f x[s, q*128+p, f]^2  ->  [SIGS, N] per core
    sq = np.empty((B, N), np.float32)
    for c in range(NCORES):
        arr = res.results[c]["norms"]            # [128, 64]
        sq[c * SIGS:(c + 1) * SIGS] = (
            arr.reshape(128, SIGS, N // 128).transpose(1, 2, 0).reshape(SIGS, N)
        )

    # D[b] = ||xi||^2 + ||xj||^2 - 2 x x^T
    G = np.matmul(data, data.transpose(0, 2, 1))  # [B, N, N] f32
    D = (sq[:, :, None] + sq[:, None, :] - np.float32(2.0) * G).astype(np.float32)

    # SoftDTW anti-diagonal DP, all B at once; grab R[L-1, L-1] per signature.
    L = np.clip(lens, 1, N).astype(np.int64)
    ii = np.arange(N)
    sdtw = np.zeros(B, np.float32)

    R0 = np.where(ii[None, :] == 0, D[:, 0, 0][:, None], BIG).astype(np.float32)
    hit = (2 * L - 2 == 0)
    if hit.any():
        sdtw[hit] = R0[hit, 0]
    Rm2 = np.full((B, N), BIG, np.float32)
    Rm1 = R0
    for k in range(1, 2 * N - 1):
        jidx = k - ii
        valid = (jidx >= 0) & (jidx < N)
        jcl = np.clip(jidx, 0, N - 1)
        Dk = np.where(valid[None, :], D[:, ii, jcl], np.float32(0.0)).astype(np.float32)
        Rk = Dk + _softmin3(_shift(Rm2), _shift(Rm1), Rm1)
        Rk = np.where(valid[None, :], Rk, BIG).astype(np.float32)
        hit = (2 * L - 2 == k)
        if hit.any():
            bs = np.nonzero(hit)[0]
            sdtw[bs] = Rk[bs, L[bs] - 1]
        Rm2, Rm1 = Rm1, Rk

    dists = (sdtw / (np.float32(2.0) * L.astype(np.float32))).astype(np.float32)

    d = dists.reshape(NW, STEP)
    dm = ((d[:, :, None] + d[:, None, :]) * np.float32(0.5)).astype(np.float32)
    g = NG + 1
    dmg = dm[:, :g, :g]
    neg = dm[:, :g, g:]
    scores = np.maximum(dmg[:, :, :, None] + MARGIN - neg[:, :, None, :], np.float32(0.0))
    maxj = scores.max(axis=(2, 3)).astype(np.float32)          # [NW, g]
    sum_lks = maxj.sum(axis=1) * np.float32(g * NF)
    nnz = (maxj != 0).astype(np.float32).sum(axis=1) * np.float32(g * NF)
    lv = sum_lks / (nnz + np.float32(1.0))
    tril = np.tril(np.ones((g, g), bool), k=-1)
    only_pos = np.where(tril[None], dmg, np.float32(0.0)).sum(axis=(1, 2)) * (
        MODEL_LAMBDA / np.float32(NG)
    )
    loss = (lv + only_pos).sum() / np.float32(NW)
    return np.float32(loss)



# revision 3
# speedup vs baseline: 24.6637x; 24.6637x over previous
"""Hard triplet loss over SoftDTW self-distances — TRN2 Bass kernel.

Algorithm (per core, 16 of the 128 signatures, data-parallel over 8 cores):

1. W production (Tensor+Act engines): the pairwise squared distance
   D[i,j] enters only through w = exp(-D/gamma).  With augmented
   vectors u_i = (2 x_i, -|x_i|^2, -1), v_j = (x_j, 1, |x_j|^2) the PE
   matmul u.v directly yields -D, and one activation computes
   w = Exp((-D) * 1/gamma) from PSUM.  Only a |j-i|<16 band is needed:
   the SoftDTW Gibbs weights decay like exp(-|D|/gamma) ~ 3e-6 per
   off-diagonal step, so the band truncation error is ~e^-200.
2. Band gather: the [i-part, j-free] tiles round-trip through a DRAM
   scratch buffer; the re-read uses a diagonal (stride 161) access
   pattern, landing W in scan-ready [sig-part, (row, delta)] layout.
3. DP (Vector engine): in probability domain P = exp(-R/gamma) the
   SoftDTW recurrence is linear:  P[i,j] = w*(P[i-1,j-1] + P[i-1,j]
   + P[i,j-1]), i.e. per row one pair-sum (tensor_tensor add) and one
   hardware scan (tensor_tensor_scan, state=(up+state)*w). 512 serial
   rows; drains guard the same-engine RAW pipeline hazard.
4. Diagonal extract + host epilogue: R(L) = -gamma*ln(P[L-1,L-1]),
   dists = R/(2L), then the tiny triplet-margin reduction in numpy.
"""
import numpy as np

import concourse.bass as bass
import concourse.mybir as mybir
from concourse.bass_utils import run_bass_kernel_spmd

NG_, NF_, NW_ = 5, 10, 8
STEP = NG_ + NF_ + 1            # 16 signatures per writer
MARGIN = np.float32(1.0)
MODEL_LAMBDA = np.float32(0.01)
GAMMA = np.float32(5.0)

B, N, F = 128, 512, 32
NCORES = 8
S = B // NCORES                 # 16 signatures per core
HB = 16                         # half band width
BW = 2 * HB                     # 32 band slots, delta = j - i + HB
SW = BW + 1                     # stored row width (slot BW is a zero pad)
NB = N // 128                   # 4 row blocks of 128
TW = 128 + BW                   # 160 j-columns produced per row block
K = F + 2                       # augmented feature dim
VW = N + BW                     # 544 padded V columns per signature
PADC = np.float32(50.0)         # pad column makes -D ~ -50*(|x|^2+1) -> w=0


def _build_core_kernel():
    nc = bass.Bass()
    vx = nc.declare_dram_parameter("vx", [F, S * VW], mybir.dt.bfloat16, isOutput=False)
    vsq = nc.declare_dram_parameter("vsq", [2, S * VW], mybir.dt.float32, isOutput=False)
    usq = nc.declare_dram_parameter("usq", [2, S * N], mybir.dt.float32, isOutput=False)
    sdiag = nc.declare_dram_parameter("sdiag", [S, N], mybir.dt.float32, isOutput=True)
    wdd = nc.dram_tensor("wdd", [S * NB * 128 * TW], mybir.dt.bfloat16)

    from contextlib import ExitStack
    with ExitStack() as es:
        UAx = es.enter_context(nc.sbuf_tensor([F, S * N], mybir.dt.bfloat16))
        UAsq = es.enter_context(nc.sbuf_tensor([2, S * N], mybir.dt.float32))
        VAx = es.enter_context(nc.sbuf_tensor([F, S * VW], mybir.dt.bfloat16))
        VAsq = es.enter_context(nc.sbuf_tensor([2, S * VW], mybir.dt.float32))
        WT0 = es.enter_context(nc.sbuf_tensor([128, TW], mybir.dt.bfloat16))
        WT1 = es.enter_context(nc.sbuf_tensor([128, TW], mybir.dt.bfloat16))
        WB = es.enter_context(nc.sbuf_tensor([S, N * BW], mybir.dt.bfloat16))
        SR = es.enter_context(nc.sbuf_tensor([S, N * SW], mybir.dt.float32))
        UP = es.enter_context(nc.sbuf_tensor([S, BW], mybir.dt.float32))
        UP0 = es.enter_context(nc.sbuf_tensor([S, BW], mybir.dt.float32))
        SD = es.enter_context(nc.sbuf_tensor([S, N], mybir.dt.float32))
        P0 = es.enter_context(nc.psum_tensor([128, TW], mybir.dt.float32))
        P1 = es.enter_context(nc.psum_tensor([128, TW], mybir.dt.float32))
        s_in = es.enter_context(nc.semaphore("s_in"))
        s_prep = es.enter_context(nc.semaphore("s_prep"))
        s_mm = es.enter_context(nc.semaphore("s_mm"))
        s_exp = es.enter_context(nc.semaphore("s_exp"))
        s_wr = es.enter_context(nc.semaphore("s_wr"))
        s_band = es.enter_context(nc.semaphore("s_band"))
        s_dp = es.enter_context(nc.semaphore("s_dp"))
        s_ext = es.enter_context(nc.semaphore("s_ext"))
        s_out = es.enter_context(nc.semaphore("s_out"))
        block = es.enter_context(nc.Block())
        WT = (WT0, WT1)
        PP = (P0, P1)

        @block.sync
        def _(sync):
            sync.dma_start(out=VAx[:, :], in_=vx[:, :]).then_inc(s_in, 16)
            sync.dma_start(out=VAsq[:, :], in_=vsq[:, :]).then_inc(s_in, 16)
            sync.dma_start(out=UAsq[:, :], in_=usq[:, :]).then_inc(s_in, 16)
            for ib in range(NB):
                for s in range(S):
                    k = ib * S + s
                    sync.wait_ge(s_exp, k + 1)
                    # plain contiguous write of the [128, TW] tile
                    sync.dma_start(
                        out=bass.AP(wdd, (s * NB + ib) * 128 * TW,
                                    [[TW, 128], [1, TW]]),
                        in_=WT[k % 2][:, :],
                    ).then_inc(s_wr, 16)
                sync.wait_ge(s_wr, 16 * S * (ib + 1))
                # diagonal band re-read: for (s, p, d):
                #   src elem = (s*NB+ib)*128*TW + p*(TW+1) + d
                sync.dma_start(
                    out=bass.AP(WB, ib * 128 * BW,
                                [[N * BW, S], [BW, 128], [1, BW]]),
                    in_=bass.AP(wdd, ib * 128 * TW,
                                [[NB * 128 * TW, S], [TW + 1, 128], [1, BW]]),
                ).then_inc(s_band, 16)
            sync.wait_ge(s_ext, 1)
            sync.dma_start(out=sdiag[:, :], in_=SD[:, :]).then_inc(s_out, 16)
            sync.wait_ge(s_out, 16)

        @block.tensor
        def _(tensor):
            tensor.wait_ge(s_in, 48)
            tensor.wait_ge(s_prep, 1)
            for ib in range(NB):
                for s in range(S):
                    k = ib * S + s
                    if k >= 2:
                        tensor.wait_ge(s_exp, k - 1)
                    tensor.matmul(
                        PP[k % 2][:, :],
                        UAx[:, s * N + ib * 128: s * N + ib * 128 + 128],
                        VAx[:, s * VW + ib * 128: s * VW + ib * 128 + TW],
                        start=True, stop=False,
                    )
                    tensor.matmul(
                        PP[k % 2][:, :],
                        UAsq[:, s * N + ib * 128: s * N + ib * 128 + 128],
                        VAsq[:, s * VW + ib * 128: s * VW + ib * 128 + TW],
                        start=False, stop=True,
                    ).then_inc(s_mm, 1)

        @block.scalar
        def _(scalar):
            # Build U rows 0..31 = 2 x^T on device from the V operand
            # (rows 32..33 = (-sq, -1) arrive by DMA).
            scalar.wait_ge(s_in, 16)
            scalar.mul(
                bass.AP(UAx, 0, [[S * N, F], [N, S], [1, N]]),
                bass.AP(VAx, HB, [[S * VW, F], [VW, S], [1, N]]),
                2.0,
            ).then_inc(s_prep, 1)
            for k in range(NB * S):
                scalar.wait_ge(s_mm, k + 1)
                if k >= 2:
                    scalar.wait_ge(s_wr, 16 * (k - 1))
                scalar.activation(
                    WT[k % 2][:, :], PP[k % 2][:, :],
                    mybir.ActivationFunctionType.Exp,
                    bias=0.0, scale=float(1.0 / GAMMA),
                ).then_inc(s_exp, 1)
            scalar.wait_ge(s_dp, 1)
            scalar.copy(SD[:, :], bass.AP(SR, HB, [[N * SW, S], [SW, N]])
                        ).then_inc(s_ext, 1)

        @block.vector
        def _(vector):
            vector.memset(SR[:, :], 0.0)
            vector.memset(UP0[:, :], 0.0)
            vector.memset(UP0[:, HB:HB + 1], 1.0)
            vector.drain()
            for ib in range(NB):
                vector.wait_ge(s_band, 16 * (ib + 1))
                for i in range(ib * 128, ib * 128 + 128):
                    if i == 0:
                        vector.tensor_tensor_scan(
                            SR[:, 0:BW], UP0[:, :], WB[:, 0:BW], 0.0,
                            mybir.AluOpType.add, mybir.AluOpType.mult)
                        continue
                    po = (i - 1) * SW
                    vector.drain()
                    vector.tensor_tensor(
                        UP[:, :], SR[:, po:po + BW], SR[:, po + 1:po + BW + 1],
                        mybir.AluOpType.add)
                    vector.drain()
                    vector.tensor_tensor_scan(
                        SR[:, i * SW:i * SW + BW], UP[:, :],
                        WB[:, i * BW:(i + 1) * BW], 0.0,
                        mybir.AluOpType.add, mybir.AluOpType.mult)
            vector.engine_nop().then_inc(s_dp, 1)

    return nc


_NC = None


def _get_nc():
    global _NC
    if _NC is None:
        _NC = _build_core_kernel()
    return _NC


def _prep_inputs(data):
    """Build per-core operands.  x ships in bf16; sq is computed FROM the
    bf16 values (in f32) so the matmul's D[i,i] cancels to ~0 exactly.
    V = (x^T; 1; sq) with w=0 pad columns, U = (2x^T; -sq; -1)."""
    import ml_dtypes
    bf16 = ml_dtypes.bfloat16
    x = data.reshape(NCORES, S, N, F)
    xb = x.astype(bf16)
    xf = xb.astype(np.float32)
    sq = np.einsum('csnf,csnf->csn', xf, xf).astype(np.float32)
    xT = xb.transpose(0, 3, 1, 2)                                 # [C, F, S, N]
    vax = np.zeros((NCORES, F, S, VW), bf16)
    vax[:, :, :, HB:HB + N] = xT
    vasq = np.full((NCORES, 2, S, VW), PADC, np.float32)
    vasq[:, 0, :, HB:HB + N] = 1.0
    vasq[:, 1, :, HB:HB + N] = sq
    usq = np.empty((NCORES, 2, S, N), np.float32)
    usq[:, 0] = -sq
    usq[:, 1] = -1.0
    return (vax.reshape(NCORES, F, S * VW),
            vasq.reshape(NCORES, 2, S * VW),
            usq.reshape(NCORES, 2, S * N))


_DISPATCH = None


def _get_dispatch():
    """Build the sharded jitted executable ONCE (run_bass_via_pjrt re-traces
    per call; this mirrors its multi-core path with a cached jit)."""
    global _DISPATCH
    if _DISPATCH is None:
        import jax
        from jax.sharding import Mesh, PartitionSpec
        from jax.experimental.shard_map import shard_map
        from concourse import bass2jax as b2j

        nc = _get_nc()
        b2j.install_neuronx_cc_hook()
        partition_name = (nc.partition_id_tensor.name
                          if nc.partition_id_tensor else None)
        in_names, out_names, out_avals = [], [], []
        out_shapes = []
        for alloc in nc.m.functions[0].allocations:
            if not isinstance(alloc, mybir.MemoryLocationSet):
                continue
            name = alloc.memorylocations[0].name
            if alloc.kind == "ExternalInput":
                if name != partition_name:
                    in_names.append(name)
            elif alloc.kind == "ExternalOutput":
                out_names.append(name)
                shape = tuple(alloc.tensor_shape)
                dtype = mybir.dt.np(alloc.dtype)
                out_avals.append(jax.core.ShapedArray(shape, dtype))
                out_shapes.append((shape, dtype))
        n_params = len(in_names)
        n_outs = len(out_names)
        bind_names = list(in_names) + list(out_names)
        if partition_name is not None:
            bind_names.append(partition_name)
        donate = tuple(range(n_params, n_params + n_outs))

        def _body(*args):
            operands = list(args)
            if partition_name is not None:
                operands.append(b2j.partition_id_tensor())
            outs = b2j._bass_exec_p.bind(
                *operands,
                out_avals=tuple(out_avals),
                in_names=tuple(bind_names),
                out_names=tuple(out_names),
                lowering_input_output_aliases=(),
                sim_require_finite=True,
                sim_require_nnan=True,
                nc=nc,
            )
            return tuple(outs)

        devices = jax.devices()[:NCORES]
        mesh = Mesh(np.asarray(devices), ("core",))
        in_specs = (PartitionSpec("core"),) * (n_params + n_outs)
        out_specs = (PartitionSpec("core"),) * n_outs
        sharded = jax.jit(
            shard_map(_body, mesh=mesh, in_specs=in_specs,
                      out_specs=out_specs, check_rep=False),
            donate_argnums=donate, keep_unused=True)
        _DISPATCH = (sharded, list(in_names), out_shapes)
    return _DISPATCH


_FAST_OK = True


def _run_device(vax, vasq, usq):
    """Fast path: cached jitted shard_map dispatch.  Falls back to the
    stock run_bass_kernel_spmd if the cached-jit internals ever break."""
    global _FAST_OK
    if _FAST_OK:
        try:
            sharded, in_names, out_shapes = _get_dispatch()
            assert in_names == ["vx", "vsq", "usq"], in_names
            concat_in = [
                np.ascontiguousarray(vax.reshape(NCORES * F, S * VW)),
                np.ascontiguousarray(vasq.reshape(NCORES * 2, S * VW)),
                np.ascontiguousarray(usq.reshape(NCORES * 2, S * N))]
            concat_zeros = [np.zeros((NCORES * shp[0],) + shp[1:], dt)
                            for shp, dt in out_shapes]
            out_arrs = sharded(*concat_in, *concat_zeros)
            return np.asarray(out_arrs[0]).reshape(B, N)
        except Exception:
            _FAST_OK = False
    nc = _get_nc()
    in_maps = [{"vx": np.ascontiguousarray(vax[c]),
                "vsq": np.ascontiguousarray(vasq[c]),
                "usq": np.ascontiguousarray(usq[c])} for c in range(NCORES)]
    res = run_bass_kernel_spmd(nc, in_maps, list(range(NCORES)))
    return np.concatenate([res.results[c]["sdiag"] for c in range(NCORES)], 0)


def kernel(data: np.ndarray, lens: np.ndarray) -> np.ndarray:
    data = np.asarray(data, np.float32)
    lens = np.asarray(lens, np.int32)

    vax, vasq, usq = _prep_inputs(data)
    sdiag = _run_device(vax, vasq, usq)

    L = np.clip(lens, 1, N).astype(np.int64)
    pll = sdiag[np.arange(B), L - 1]
    R = (-GAMMA * np.log(np.maximum(pll, np.float32(1e-30)))).astype(np.float32)
    dists = (R / (np.float32(2.0) * L.astype(np.float32))).astype(np.float32)

    d = dists.reshape(NW_, STEP)
    dm = ((d[:, :, None] + d[:, None, :]) * np.float32(0.5)).astype(np.float32)
    g = NG_ + 1
    dmg = dm[:, :g, :g]
    neg = dm[:, :g, g:]
    scores = np.maximum(dmg[:, :, :, None] + MARGIN - neg[:, :, None, :],
                        np.float32(0.0))
    maxj = scores.max(axis=(2, 3)).astype(np.float32)
    sum_lks = maxj.sum(axis=1) * np.float32(g * NF_)
    nnz = (maxj != 0).astype(np.float32).sum(axis=1) * np.float32(g * NF_)
    lv = sum_lks / (nnz + np.float32(1.0))
    tril = np.tril(np.ones((g, g), bool), k=-1)
    only_pos = np.where(tril[None], dmg, np.float32(0.0)).sum(axis=(1, 2)) * (
        MODEL_LAMBDA / np.float32(NG_))
    loss = (lv + only_pos).sum() / np.float32(NW_)
    return np.float32(loss)


# revision 4
# speedup vs baseline: 31.1055x; 1.2612x over previous
"""Hard triplet loss over SoftDTW self-distances — TRN2 Bass kernel.

Algorithm (per core, 16 of the 128 signatures, data-parallel over 8 cores):

1. W production (Tensor+Act engines): the pairwise squared distance
   D[i,j] enters only through w = exp(-D/gamma).  With augmented
   vectors u_i = (2 x_i, -|x_i|^2, -1), v_j = (x_j, 1, |x_j|^2) the PE
   matmul u.v directly yields -D, and one activation computes
   w = Exp((-D) * 1/gamma) from PSUM.  Only a |j-i|<16 band is needed:
   the SoftDTW Gibbs weights decay like exp(-|D|/gamma) ~ 3e-6 per
   off-diagonal step, so the band truncation error is ~e^-200.
2. Band gather: the [i-part, j-free] tiles round-trip through a DRAM
   scratch buffer; the re-read uses a diagonal (stride 161) access
   pattern, landing W in scan-ready [sig-part, (row, delta)] layout.
3. DP (Vector engine): in probability domain P = exp(-R/gamma) the
   SoftDTW recurrence is linear:  P[i,j] = w*(P[i-1,j-1] + P[i-1,j]
   + P[i,j-1]), i.e. per row one pair-sum (tensor_tensor add) and one
   hardware scan (tensor_tensor_scan, state=(up+state)*w). 512 serial
   rows; drains guard the same-engine RAW pipeline hazard.
4. Diagonal extract + host epilogue: R(L) = -gamma*ln(P[L-1,L-1]),
   dists = R/(2L), then the tiny triplet-margin reduction in numpy.
"""
import numpy as np

import concourse.bass as bass
import concourse.mybir as mybir
from concourse.bass_utils import run_bass_kernel_spmd

NG_, NF_, NW_ = 5, 10, 8
STEP = NG_ + NF_ + 1            # 16 signatures per writer
MARGIN = np.float32(1.0)
MODEL_LAMBDA = np.float32(0.01)
GAMMA = np.float32(5.0)

B, N, F = 128, 512, 32
NCORES = 8
S = B // NCORES                 # 16 signatures per core
HB = 16                         # half band width
BW = 2 * HB                     # 32 band slots, delta = j - i + HB
SW = BW + 1                     # stored row width (slot BW is a zero pad)
NB = N // 128                   # 4 row blocks of 128
TW = 128 + BW                   # 160 j-columns produced per row block
K = F + 2                       # augmented feature dim
VW = N + BW                     # 544 padded V columns per signature
PADC = np.float32(50.0)         # pad column makes -D ~ -50*(|x|^2+1) -> w=0


def _build_core_kernel():
    nc = bass.Bass()
    vx = nc.declare_dram_parameter("vx", [F, S * VW], mybir.dt.bfloat16, isOutput=False)
    vsq = nc.declare_dram_parameter("vsq", [2, S * VW], mybir.dt.float32, isOutput=False)
    usq = nc.declare_dram_parameter("usq", [2, S * N], mybir.dt.float32, isOutput=False)
    sdiag = nc.declare_dram_parameter("sdiag", [S, N], mybir.dt.float32, isOutput=True)
    wdd = nc.dram_tensor("wdd", [S * NB * 128 * TW], mybir.dt.bfloat16)

    from contextlib import ExitStack
    with ExitStack() as es:
        UAx = es.enter_context(nc.sbuf_tensor([F, S * N], mybir.dt.bfloat16))
        UAsq = es.enter_context(nc.sbuf_tensor([2, S * N], mybir.dt.float32))
        VAx = es.enter_context(nc.sbuf_tensor([F, S * VW], mybir.dt.bfloat16))
        VAsq = es.enter_context(nc.sbuf_tensor([2, S * VW], mybir.dt.float32))
        WT0 = es.enter_context(nc.sbuf_tensor([128, TW], mybir.dt.bfloat16))
        WT1 = es.enter_context(nc.sbuf_tensor([128, TW], mybir.dt.bfloat16))
        WB = es.enter_context(nc.sbuf_tensor([S, N * BW], mybir.dt.bfloat16))
        SR = es.enter_context(nc.sbuf_tensor([S, N * SW], mybir.dt.float32))
        UP = es.enter_context(nc.sbuf_tensor([S, BW], mybir.dt.float32))
        UP0 = es.enter_context(nc.sbuf_tensor([S, BW], mybir.dt.float32))
        SD = es.enter_context(nc.sbuf_tensor([S, N], mybir.dt.float32))
        P0 = es.enter_context(nc.psum_tensor([128, TW], mybir.dt.float32))
        P1 = es.enter_context(nc.psum_tensor([128, TW], mybir.dt.float32))
        s_in = es.enter_context(nc.semaphore("s_in"))
        s_prep = es.enter_context(nc.semaphore("s_prep"))
        s_mm = es.enter_context(nc.semaphore("s_mm"))
        s_exp = es.enter_context(nc.semaphore("s_exp"))
        s_wr = es.enter_context(nc.semaphore("s_wr"))
        s_band = es.enter_context(nc.semaphore("s_band"))
        s_dp = es.enter_context(nc.semaphore("s_dp"))
        s_ext = es.enter_context(nc.semaphore("s_ext"))
        s_out = es.enter_context(nc.semaphore("s_out"))
        block = es.enter_context(nc.Block())
        WT = (WT0, WT1)
        PP = (P0, P1)

        @block.sync
        def _(sync):
            sync.dma_start(out=VAx[:, :], in_=vx[:, :]).then_inc(s_in, 16)
            sync.dma_start(out=VAsq[:, :], in_=vsq[:, :]).then_inc(s_in, 16)
            sync.dma_start(out=UAsq[:, :], in_=usq[:, :]).then_inc(s_in, 16)
            for ib in range(NB):
                for s in range(S):
                    k = ib * S + s
                    sync.wait_ge(s_exp, k + 1)
                    # plain contiguous write of the [128, TW] tile
                    sync.dma_start(
                        out=bass.AP(wdd, (s * NB + ib) * 128 * TW,
                                    [[TW, 128], [1, TW]]),
                        in_=WT[k % 2][:, :],
                    ).then_inc(s_wr, 16)
                sync.wait_ge(s_wr, 16 * S * (ib + 1))
                # diagonal band re-read: for (s, p, d):
                #   src elem = (s*NB+ib)*128*TW + p*(TW+1) + d
                sync.dma_start(
                    out=bass.AP(WB, ib * 128 * BW,
                                [[N * BW, S], [BW, 128], [1, BW]]),
                    in_=bass.AP(wdd, ib * 128 * TW,
                                [[NB * 128 * TW, S], [TW + 1, 128], [1, BW]]),
                ).then_inc(s_band, 16)
            sync.wait_ge(s_ext, 1)
            sync.dma_start(out=sdiag[:, :], in_=SD[:, :]).then_inc(s_out, 16)
            sync.wait_ge(s_out, 16)

        @block.tensor
        def _(tensor):
            tensor.wait_ge(s_in, 48)
            tensor.wait_ge(s_prep, 1)
            for ib in range(NB):
                for s in range(S):
                    k = ib * S + s
                    if k >= 2:
                        tensor.wait_ge(s_exp, k - 1)
                    tensor.matmul(
                        PP[k % 2][:, :],
                        UAx[:, s * N + ib * 128: s * N + ib * 128 + 128],
                        VAx[:, s * VW + ib * 128: s * VW + ib * 128 + TW],
                        start=True, stop=False,
                    )
                    tensor.matmul(
                        PP[k % 2][:, :],
                        UAsq[:, s * N + ib * 128: s * N + ib * 128 + 128],
                        VAsq[:, s * VW + ib * 128: s * VW + ib * 128 + TW],
                        start=False, stop=True,
                    ).then_inc(s_mm, 1)

        @block.scalar
        def _(scalar):
            # Build U rows 0..31 = 2 x^T on device from the V operand
            # (rows 32..33 = (-sq, -1) arrive by DMA).  Wait for ALL input
            # DMAs: completions may land out of issue order across queues.
            scalar.wait_ge(s_in, 48)
            scalar.mul(
                bass.AP(UAx, 0, [[S * N, F], [N, S], [1, N]]),
                bass.AP(VAx, HB, [[S * VW, F], [VW, S], [1, N]]),
                2.0,
            ).then_inc(s_prep, 1)
            for k in range(NB * S):
                scalar.wait_ge(s_mm, k + 1)
                if k >= 2:
                    scalar.wait_ge(s_wr, 16 * (k - 1))
                scalar.activation(
                    WT[k % 2][:, :], PP[k % 2][:, :],
                    mybir.ActivationFunctionType.Exp,
                    bias=0.0, scale=float(1.0 / GAMMA),
                ).then_inc(s_exp, 1)
            scalar.wait_ge(s_dp, 1)
            scalar.copy(SD[:, :], bass.AP(SR, HB, [[N * SW, S], [SW, N]])
                        ).then_inc(s_ext, 1)

        @block.vector
        def _(vector):
            vector.memset(SR[:, :], 0.0)
            vector.memset(UP0[:, :], 0.0)
            vector.memset(UP0[:, HB:HB + 1], 1.0)
            vector.drain()
            for ib in range(NB):
                vector.wait_ge(s_band, 16 * (ib + 1))
                for i in range(ib * 128, ib * 128 + 128):
                    if i == 0:
                        vector.tensor_tensor_scan(
                            SR[:, 0:BW], UP0[:, :], WB[:, 0:BW], 0.0,
                            mybir.AluOpType.add, mybir.AluOpType.mult)
                        continue
                    po = (i - 1) * SW
                    vector.drain()
                    vector.tensor_tensor(
                        UP[:, :], SR[:, po:po + BW], SR[:, po + 1:po + BW + 1],
                        mybir.AluOpType.add)
                    vector.drain()
                    vector.tensor_tensor_scan(
                        SR[:, i * SW:i * SW + BW], UP[:, :],
                        WB[:, i * BW:(i + 1) * BW], 0.0,
                        mybir.AluOpType.add, mybir.AluOpType.mult)
            vector.engine_nop().then_inc(s_dp, 1)

    return nc


_NC = None


def _get_nc():
    global _NC
    if _NC is None:
        _NC = _build_core_kernel()
    return _NC


def _prep_inputs(data):
    """Build per-core operands.  x ships in bf16; sq is computed FROM the
    bf16 values (in f32) so the matmul's D[i,i] cancels to ~0 exactly.
    V = (x^T; 1; sq) with w=0 pad columns, U = (2x^T; -sq; -1)."""
    import ml_dtypes
    bf16 = ml_dtypes.bfloat16
    x = data.reshape(NCORES, S, N, F)
    xb = x.astype(bf16)
    xf = xb.astype(np.float32)
    sq = np.einsum('csnf,csnf->csn', xf, xf).astype(np.float32)
    xT = xb.transpose(0, 3, 1, 2)                                 # [C, F, S, N]
    vax = np.zeros((NCORES, F, S, VW), bf16)
    vax[:, :, :, HB:HB + N] = xT
    vasq = np.full((NCORES, 2, S, VW), PADC, np.float32)
    vasq[:, 0, :, HB:HB + N] = 1.0
    vasq[:, 1, :, HB:HB + N] = sq
    usq = np.empty((NCORES, 2, S, N), np.float32)
    usq[:, 0] = -sq
    usq[:, 1] = -1.0
    return (vax.reshape(NCORES, F, S * VW),
            vasq.reshape(NCORES, 2, S * VW),
            usq.reshape(NCORES, 2, S * N))


_DISPATCH = None


def _get_dispatch():
    """Build the sharded jitted executable ONCE (run_bass_via_pjrt re-traces
    per call; this mirrors its multi-core path with a cached jit)."""
    global _DISPATCH
    if _DISPATCH is None:
        import jax
        from jax.sharding import Mesh, PartitionSpec
        from jax.experimental.shard_map import shard_map
        from concourse import bass2jax as b2j

        nc = _get_nc()
        b2j.install_neuronx_cc_hook()
        partition_name = (nc.partition_id_tensor.name
                          if nc.partition_id_tensor else None)
        in_names, out_names, out_avals = [], [], []
        out_shapes = []
        for alloc in nc.m.functions[0].allocations:
            if not isinstance(alloc, mybir.MemoryLocationSet):
                continue
            name = alloc.memorylocations[0].name
            if alloc.kind == "ExternalInput":
                if name != partition_name:
                    in_names.append(name)
            elif alloc.kind == "ExternalOutput":
                out_names.append(name)
                shape = tuple(alloc.tensor_shape)
                dtype = mybir.dt.np(alloc.dtype)
                out_avals.append(jax.core.ShapedArray(shape, dtype))
                out_shapes.append((shape, dtype))
        n_params = len(in_names)
        n_outs = len(out_names)
        bind_names = list(in_names) + list(out_names)
        if partition_name is not None:
            bind_names.append(partition_name)
        donate = tuple(range(n_params, n_params + n_outs))

        def _body(*args):
            operands = list(args)
            if partition_name is not None:
                operands.append(b2j.partition_id_tensor())
            outs = b2j._bass_exec_p.bind(
                *operands,
                out_avals=tuple(out_avals),
                in_names=tuple(bind_names),
                out_names=tuple(out_names),
                lowering_input_output_aliases=(),
                sim_require_finite=True,
                sim_require_nnan=True,
                nc=nc,
            )
            return tuple(outs)

        devices = jax.devices()[:NCORES]
        mesh = Mesh(np.asarray(devices), ("core",))
        in_specs = (PartitionSpec("core"),) * (n_params + n_outs)
        out_specs = (PartitionSpec("core"),) * n_outs
        sharded = jax.jit(
            shard_map(_body, mesh=mesh, in_specs=in_specs,
                      out_specs=out_specs, check_rep=False),
            donate_argnums=donate, keep_unused=True)
        _DISPATCH = (sharded, list(in_names), out_shapes)
    return _DISPATCH


_FAST_OK = True


def _run_device(vax, vasq, usq):
    """Fast path: cached jitted shard_map dispatch.  Falls back to the
    stock run_bass_kernel_spmd if the cached-jit internals ever break."""
    global _FAST_OK
    if _FAST_OK:
        try:
            sharded, in_names, out_shapes = _get_dispatch()
            assert in_names == ["vx", "vsq", "usq"], in_names
            concat_in = [
                np.ascontiguousarray(vax.reshape(NCORES * F, S * VW)),
                np.ascontiguousarray(vasq.reshape(NCORES * 2, S * VW)),
                np.ascontiguousarray(usq.reshape(NCORES * 2, S * N))]
            concat_zeros = [np.zeros((NCORES * shp[0],) + shp[1:], dt)
                            for shp, dt in out_shapes]
            out_arrs = sharded(*concat_in, *concat_zeros)
            return np.asarray(out_arrs[0]).reshape(B, N)
        except Exception:
            _FAST_OK = False
    nc = _get_nc()
    in_maps = [{"vx": np.ascontiguousarray(vax[c]),
                "vsq": np.ascontiguousarray(vasq[c]),
                "usq": np.ascontiguousarray(usq[c])} for c in range(NCORES)]
    res = run_bass_kernel_spmd(nc, in_maps, list(range(NCORES)))
    return np.concatenate([res.results[c]["sdiag"] for c in range(NCORES)], 0)


def kernel(data: np.ndarray, lens: np.ndarray) -> np.ndarray:
    data = np.asarray(data, np.float32)
    lens = np.asarray(lens, np.int32)

    vax, vasq, usq = _prep_inputs(data)
    sdiag = _run_device(vax, vasq, usq)

    L = np.clip(lens, 1, N).astype(np.int64)
    pll = sdiag[np.arange(B), L - 1]
    R = (-GAMMA * np.log(np.maximum(pll, np.float32(1e-30)))).astype(np.float32)
    dists = (R / (np.float32(2.0) * L.astype(np.float32))).astype(np.float32)

    d = dists.reshape(NW_, STEP)
    dm = ((d[:, :, None] + d[:, None, :]) * np.float32(0.5)).astype(np.float32)
    g = NG_ + 1
    dmg = dm[:, :g, :g]
    neg = dm[:, :g, g:]
    scores = np.maximum(dmg[:, :, :, None] + MARGIN - neg[:, :, None, :],
                        np.float32(0.0))
    maxj = scores.max(axis=(2, 3)).astype(np.float32)
    sum_lks = maxj.sum(axis=1) * np.float32(g * NF_)
    nnz = (maxj != 0).astype(np.float32).sum(axis=1) * np.float32(g * NF_)
    lv = sum_lks / (nnz + np.float32(1.0))
    tril = np.tril(np.ones((g, g), bool), k=-1)
    only_pos = np.where(tril[None], dmg, np.float32(0.0)).sum(axis=(1, 2)) * (
        MODEL_LAMBDA / np.float32(NG_))
    loss = (lv + only_pos).sum() / np.float32(NW_)
    return np.float32(loss)


# revision 7
# speedup vs baseline: 38.6696x; 1.2432x over previous
"""Hard triplet loss over SoftDTW self-distances — TRN2 Bass kernel.

Algorithm (per core, 16 of the 128 signatures, data-parallel over 8 cores):

1. W production (Tensor+Act engines): the pairwise squared distance
   D[i,j] enters only through w = exp(-D/gamma).  With augmented
   vectors u_i = (2 x_i, -|x_i|^2, -1), v_j = (x_j, 1, |x_j|^2) the PE
   matmul u.v directly yields -D, and one activation computes
   w = Exp((-D) * 1/gamma) from PSUM.  Only a |j-i|<16 band is needed:
   the SoftDTW Gibbs weights decay like exp(-|D|/gamma) ~ 3e-6 per
   off-diagonal step, so the band truncation error is ~e^-200.
2. Band gather: the [i-part, j-free] tiles round-trip through a DRAM
   scratch buffer; the re-read uses a diagonal (stride 161) access
   pattern, landing W in scan-ready [sig-part, (row, delta)] layout.
3. DP (Vector engine): in probability domain P = exp(-R/gamma) the
   SoftDTW recurrence is linear:  P[i,j] = w*(P[i-1,j-1] + P[i-1,j]
   + P[i,j-1]), i.e. per row one pair-sum (tensor_tensor add) and one
   hardware scan (tensor_tensor_scan, state=(up+state)*w). 512 serial
   rows; drains guard the same-engine RAW pipeline hazard.
4. Diagonal extract + host epilogue: R(L) = -gamma*ln(P[L-1,L-1]),
   dists = R/(2L), then the tiny triplet-margin reduction in numpy.
"""
import numpy as np

import concourse.bass as bass
import concourse.mybir as mybir
from concourse.bass_utils import run_bass_kernel_spmd

NG_, NF_, NW_ = 5, 10, 8
STEP = NG_ + NF_ + 1            # 16 signatures per writer
MARGIN = np.float32(1.0)
MODEL_LAMBDA = np.float32(0.01)
GAMMA = np.float32(5.0)

B, N, F = 128, 512, 32
NCORES = 8
S = B // NCORES                 # 16 signatures per core
HB = 16                         # half band width
BW = 2 * HB                     # 32 band slots, delta = j - i + HB
SW = BW + 1                     # stored row width (slot BW is a zero pad)
NB = N // 128                   # 4 row blocks of 128
TW = 128 + BW                   # 160 j-columns produced per row block
K = F + 2                       # augmented feature dim
VW = N + BW                     # 544 padded V columns per signature
PADC = np.float32(50.0)         # pad column makes -D ~ -50*(|x|^2+1) -> w=0


def _build_core_kernel():
    nc = bass.Bass()
    vx = nc.declare_dram_parameter("vx", [F, S * N], mybir.dt.bfloat16, isOutput=False)
    sdiag = nc.declare_dram_parameter("sdiag", [S, N], mybir.dt.float32, isOutput=True)
    wdd = nc.dram_tensor("wdd", [S * NB * 128 * TW], mybir.dt.bfloat16)
    # NEFF-embedded constants (no per-call upload): VAsq row0 = ones/PADC
    # complete; row1 = PADC (sq lands later via DMA).  UAsq row1 = -1.
    cva = np.full((2, S * VW), PADC, np.float32)
    ones_pads = np.full((S, VW), PADC, np.float32)
    ones_pads[:, HB:HB + N] = 1.0
    cva[0] = ones_pads.reshape(-1)
    c_vasq = nc.inline_tensor(cva, "c_vasq")
    cua = np.zeros((2, S * N), np.float32)
    cua[1] = -1.0
    c_uasq = nc.inline_tensor(cua, "c_uasq")

    from contextlib import ExitStack
    with ExitStack() as es:
        UAx = es.enter_context(nc.sbuf_tensor([F, S * N], mybir.dt.bfloat16))
        UAsq = es.enter_context(nc.sbuf_tensor([2, S * N], mybir.dt.float32))
        VAx = es.enter_context(nc.sbuf_tensor([F, S * VW], mybir.dt.bfloat16))
        VAsq = es.enter_context(nc.sbuf_tensor([2, S * VW], mybir.dt.float32))
        WT0 = es.enter_context(nc.sbuf_tensor([128, TW], mybir.dt.bfloat16))
        WT1 = es.enter_context(nc.sbuf_tensor([128, TW], mybir.dt.bfloat16))
        WB = es.enter_context(nc.sbuf_tensor([S, N * BW], mybir.dt.bfloat16))
        SR = es.enter_context(nc.sbuf_tensor([S, N * SW], mybir.dt.float32))
        UP = es.enter_context(nc.sbuf_tensor([S, BW], mybir.dt.float32))
        UP0 = es.enter_context(nc.sbuf_tensor([S, BW], mybir.dt.float32))
        SD = es.enter_context(nc.sbuf_tensor([S, N], mybir.dt.float32))
        XQ0 = es.enter_context(nc.sbuf_tensor([F, N], mybir.dt.float32))
        XQ1 = es.enter_context(nc.sbuf_tensor([F, N], mybir.dt.float32))
        STG = es.enter_context(nc.sbuf_tensor([1, N], mybir.dt.float32))
        ONE = es.enter_context(nc.sbuf_tensor([F, 1], mybir.dt.float32))
        P0 = es.enter_context(nc.psum_tensor([128, TW], mybir.dt.float32))
        P1 = es.enter_context(nc.psum_tensor([128, TW], mybir.dt.float32))
        PQ = es.enter_context(nc.psum_tensor([1, N], mybir.dt.float32))
        s_in = es.enter_context(nc.semaphore("s_in"))
        s_ms = es.enter_context(nc.semaphore("s_ms"))
        s_xsq = es.enter_context(nc.semaphore("s_xsq"))
        s_mmq = es.enter_context(nc.semaphore("s_mmq"))
        s_sq = es.enter_context(nc.semaphore("s_sq"))
        s_sqd = es.enter_context(nc.semaphore("s_sqd"))
        s_prep = es.enter_context(nc.semaphore("s_prep"))
        s_mm = es.enter_context(nc.semaphore("s_mm"))
        s_exp = es.enter_context(nc.semaphore("s_exp"))
        s_wr = es.enter_context(nc.semaphore("s_wr"))
        s_band = es.enter_context(nc.semaphore("s_band"))
        s_dp = es.enter_context(nc.semaphore("s_dp"))
        s_ext = es.enter_context(nc.semaphore("s_ext"))
        s_out = es.enter_context(nc.semaphore("s_out"))
        block = es.enter_context(nc.Block())
        WT = (WT0, WT1)
        PP = (P0, P1)

        @block.sync
        def _(sync):
            # unpadded x lands in the non-pad columns of VAx
            sync.dma_start(
                out=bass.AP(VAx, HB, [[S * VW, F], [VW, S], [1, N]]),
                in_=bass.AP(vx, 0, [[S * N, F], [N, S], [1, N]]),
            ).then_inc(s_in, 16)
            sync.dma_start(out=VAsq[:, :], in_=c_vasq[:, :]).then_inc(s_in, 16)
            sync.dma_start(out=UAsq[:, :], in_=c_uasq[:, :]).then_inc(s_in, 16)
            # device-computed sq rows -> VAsq row 1, one sig at a time
            # (engines cannot write partition 1; SBUF->SBUF DMAs can)
            for s in range(S):
                sync.wait_ge(s_sq, 2 * s + 1)
                sync.dma_start(
                    out=VAsq[1:2, s * VW + HB: s * VW + HB + N],
                    in_=STG[:, :],
                ).then_inc(s_sqd, 16)
            for ib in range(NB):
                for s in range(S):
                    k = ib * S + s
                    sync.wait_ge(s_exp, k + 1)
                    # plain contiguous write of the [128, TW] tile
                    sync.dma_start(
                        out=bass.AP(wdd, (s * NB + ib) * 128 * TW,
                                    [[TW, 128], [1, TW]]),
                        in_=WT[k % 2][:, :],
                    ).then_inc(s_wr, 16)
                sync.wait_ge(s_wr, 16 * S * (ib + 1))
                # diagonal band re-read: for (s, p, d):
                #   src elem = (s*NB+ib)*128*TW + p*(TW+1) + d
                sync.dma_start(
                    out=bass.AP(WB, ib * 128 * BW,
                                [[N * BW, S], [BW, 128], [1, BW]]),
                    in_=bass.AP(wdd, ib * 128 * TW,
                                [[NB * 128 * TW, S], [TW + 1, 128], [1, BW]]),
                ).then_inc(s_band, 16)
            sync.wait_ge(s_ext, 1)
            sync.dma_start(out=sdiag[:, :], in_=SD[:, :]).then_inc(s_out, 16)
            sync.wait_ge(s_out, 16)

        @block.tensor
        def _(tensor):
            tensor.wait_ge(s_ms, 3)
            # per-sig sq reduction: PQ[0, :] = sum_d XQ[d, :]
            for s in range(S):
                tensor.wait_ge(s_xsq, s + 1)
                if s >= 1:
                    tensor.wait_ge(s_sq, 2 * s)
                tensor.matmul(PQ[:, :], ONE[:, :], (XQ0 if s % 2 == 0 else XQ1)[:, :],
                              start=True, stop=True).then_inc(s_mmq, 1)
            tensor.wait_ge(s_prep, 1)
            tensor.wait_ge(s_sqd, 16 * S)
            for ib in range(NB):
                for s in range(S):
                    k = ib * S + s
                    if k >= 2:
                        tensor.wait_ge(s_exp, k - 1)
                    tensor.matmul(
                        PP[k % 2][:, :],
                        UAx[:, s * N + ib * 128: s * N + ib * 128 + 128],
                        VAx[:, s * VW + ib * 128: s * VW + ib * 128 + TW],
                        start=True, stop=False,
                    )
                    tensor.matmul(
                        PP[k % 2][:, :],
                        UAsq[:, s * N + ib * 128: s * N + ib * 128 + 128],
                        VAsq[:, s * VW + ib * 128: s * VW + ib * 128 + TW],
                        start=False, stop=True,
                    ).then_inc(s_mm, 1)

        @block.scalar
        def _(scalar):
            # Build U rows 0..31 = 2 x^T on device from the V operand.
            # Wait for ALL input DMAs: completions may land out of issue
            # order across queues.
            scalar.wait_ge(s_in, 48)
            scalar.mul(
                bass.AP(UAx, 0, [[S * N, F], [N, S], [1, N]]),
                bass.AP(VAx, HB, [[S * VW, F], [VW, S], [1, N]]),
                2.0,
            ).then_inc(s_prep, 1)
            # per-sig: square x (f32), then stage sq and -sq rows
            for s in range(S):
                if s >= 2:
                    scalar.wait_ge(s_mmq, s - 1)
                scalar.activation(
                    (XQ0 if s % 2 == 0 else XQ1)[:, :],
                    VAx[0:F, s * VW + HB: s * VW + HB + N],
                    mybir.ActivationFunctionType.Square,
                ).then_inc(s_xsq, 1)
                scalar.wait_ge(s_mmq, s + 1)
                if s >= 1:
                    scalar.wait_ge(s_sqd, 16 * s)
                scalar.copy(STG[:, :], PQ[:, :]).then_inc(s_sq, 1)
                scalar.mul(UAsq[0:1, s * N:(s + 1) * N], PQ[:, :], -1.0
                           ).then_inc(s_sq, 1)
            for k in range(NB * S):
                scalar.wait_ge(s_mm, k + 1)
                if k >= 2:
                    scalar.wait_ge(s_wr, 16 * (k - 1))
                scalar.activation(
                    WT[k % 2][:, :], PP[k % 2][:, :],
                    mybir.ActivationFunctionType.Exp,
                    bias=0.0, scale=float(1.0 / GAMMA),
                ).then_inc(s_exp, 1)
            scalar.wait_ge(s_dp, 1)
            scalar.copy(SD[:, :], bass.AP(SR, HB, [[N * SW, S], [SW, N]])
                        ).then_inc(s_ext, 1)

        @block.vector
        def _(vector):
            vector.memset(bass.AP(VAx, 0, [[S * VW, F], [VW, S], [1, HB]]), 0.0
                          ).then_inc(s_ms, 1)
            vector.memset(bass.AP(VAx, HB + N, [[S * VW, F], [VW, S], [1, HB]]), 0.0
                          ).then_inc(s_ms, 1)
            vector.memset(ONE[:, :], 1.0).then_inc(s_ms, 1)
            vector.memset(SR[:, :], 0.0)
            vector.memset(UP0[:, :], 0.0)
            vector.memset(UP0[:, HB:HB + 1], 1.0)
            vector.drain()
            for ib in range(NB):
                vector.wait_ge(s_band, 16 * (ib + 1))
                for i in range(ib * 128, ib * 128 + 128):
                    if i == 0:
                        vector.tensor_tensor_scan(
                            SR[:, 0:BW], UP0[:, :], WB[:, 0:BW], 0.0,
                            mybir.AluOpType.add, mybir.AluOpType.mult)
                        continue
                    po = (i - 1) * SW
                    vector.drain()
                    vector.tensor_tensor(
                        UP[:, :], SR[:, po:po + BW], SR[:, po + 1:po + BW + 1],
                        mybir.AluOpType.add)
                    vector.drain()
                    vector.tensor_tensor_scan(
                        SR[:, i * SW:i * SW + BW], UP[:, :],
                        WB[:, i * BW:(i + 1) * BW], 0.0,
                        mybir.AluOpType.add, mybir.AluOpType.mult)
            vector.engine_nop().then_inc(s_dp, 1)

    return nc


_NC = None


def _get_nc():
    global _NC
    if _NC is None:
        _NC = _build_core_kernel()
    return _NC


def _prep_inputs(data):
    """Only x ships (unpadded, bf16); the device derives everything else.
    sq is computed on device FROM the bf16 values (in f32) so the
    matmul's D[i,i] cancels to ~0 exactly."""
    import ml_dtypes
    x = data.reshape(NCORES, S, N, F)
    xT = x.astype(ml_dtypes.bfloat16).transpose(0, 3, 1, 2)       # [C, F, S, N]
    return np.ascontiguousarray(xT.reshape(NCORES, F, S * N))


_DISPATCH = None


def _get_dispatch():
    """Build the sharded jitted executable ONCE (run_bass_via_pjrt re-traces
    per call; this mirrors its multi-core path with a cached jit)."""
    global _DISPATCH
    if _DISPATCH is None:
        import jax
        from jax.sharding import Mesh, PartitionSpec
        from jax.experimental.shard_map import shard_map
        from concourse import bass2jax as b2j

        nc = _get_nc()
        b2j.install_neuronx_cc_hook()
        partition_name = (nc.partition_id_tensor.name
                          if nc.partition_id_tensor else None)
        in_names, out_names, out_avals = [], [], []
        out_shapes = []
        for alloc in nc.m.functions[0].allocations:
            if not isinstance(alloc, mybir.MemoryLocationSet):
                continue
            name = alloc.memorylocations[0].name
            if alloc.kind == "ExternalInput":
                if name != partition_name:
                    in_names.append(name)
            elif alloc.kind == "ExternalOutput":
                out_names.append(name)
                shape = tuple(alloc.tensor_shape)
                dtype = mybir.dt.np(alloc.dtype)
                out_avals.append(jax.core.ShapedArray(shape, dtype))
                out_shapes.append((shape, dtype))
        n_params = len(in_names)
        n_outs = len(out_names)
        bind_names = list(in_names) + list(out_names)
        if partition_name is not None:
            bind_names.append(partition_name)
        donate = tuple(range(n_params, n_params + n_outs))

        def _body(*args):
            operands = list(args)
            if partition_name is not None:
                operands.append(b2j.partition_id_tensor())
            outs = b2j._bass_exec_p.bind(
                *operands,
                out_avals=tuple(out_avals),
                in_names=tuple(bind_names),
                out_names=tuple(out_names),
                lowering_input_output_aliases=(),
                sim_require_finite=True,
                sim_require_nnan=True,
                nc=nc,
            )
            return tuple(outs)

        devices = jax.devices()[:NCORES]
        mesh = Mesh(np.asarray(devices), ("core",))
        in_specs = (PartitionSpec("core"),) * (n_params + n_outs)
        out_specs = (PartitionSpec("core"),) * n_outs
        sharded = jax.jit(
            shard_map(_body, mesh=mesh, in_specs=in_specs,
                      out_specs=out_specs, check_rep=False),
            donate_argnums=donate, keep_unused=True)
        _DISPATCH = (sharded, list(in_names), out_shapes)
    return _DISPATCH


_FAST_OK = True


def _run_device(vax):
    """Fast path: cached jitted shard_map dispatch.  Falls back to the
    stock run_bass_kernel_spmd if the cached-jit internals ever break."""
    global _FAST_OK
    if _FAST_OK:
        try:
            sharded, in_names, out_shapes = _get_dispatch()
            assert in_names == ["vx"], in_names
            concat_in = [np.ascontiguousarray(vax.reshape(NCORES * F, S * N))]
            concat_zeros = [np.zeros((NCORES * shp[0],) + shp[1:], dt)
                            for shp, dt in out_shapes]
            out_arrs = sharded(*concat_in, *concat_zeros)
            return np.asarray(out_arrs[0]).reshape(B, N)
        except Exception:
            _FAST_OK = False
    nc = _get_nc()
    in_maps = [{"vx": np.ascontiguousarray(vax[c])} for c in range(NCORES)]
    res = run_bass_kernel_spmd(nc, in_maps, list(range(NCORES)))
    return np.concatenate([res.results[c]["sdiag"] for c in range(NCORES)], 0)


def kernel(data: np.ndarray, lens: np.ndarray) -> np.ndarray:
    data = np.asarray(data, np.float32)
    lens = np.asarray(lens, np.int32)

    vax = _prep_inputs(data)
    sdiag = _run_device(vax)

    L = np.clip(lens, 1, N).astype(np.int64)
    pll = sdiag[np.arange(B), L - 1]
    R = (-GAMMA * np.log(np.maximum(pll, np.float32(1e-30)))).astype(np.float32)
    dists = (R / (np.float32(2.0) * L.astype(np.float32))).astype(np.float32)

    d = dists.reshape(NW_, STEP)
    dm = ((d[:, :, None] + d[:, None, :]) * np.float32(0.5)).astype(np.float32)
    g = NG_ + 1
    dmg = dm[:, :g, :g]
    neg = dm[:, :g, g:]
    scores = np.maximum(dmg[:, :, :, None] + MARGIN - neg[:, :, None, :],
                        np.float32(0.0))
    maxj = scores.max(axis=(2, 3)).astype(np.float32)
    sum_lks = maxj.sum(axis=1) * np.float32(g * NF_)
    nnz = (maxj != 0).astype(np.float32).sum(axis=1) * np.float32(g * NF_)
    lv = sum_lks / (nnz + np.float32(1.0))
    tril = np.tril(np.ones((g, g), bool), k=-1)
    only_pos = np.where(tril[None], dmg, np.float32(0.0)).sum(axis=(1, 2)) * (
        MODEL_LAMBDA / np.float32(NG_))
    loss = (lv + only_pos).sum() / np.float32(NW_)
    return np.float32(loss)


# revision 8
# speedup vs baseline: 42.5138x; 1.0994x over previous
"""Hard triplet loss over SoftDTW self-distances — TRN2 Bass kernel.

Algorithm (per core, 16 of the 128 signatures, data-parallel over 8 cores):

1. W production (Tensor+Act engines): the pairwise squared distance
   D[i,j] enters only through w = exp(-D/gamma).  With augmented
   vectors u_i = (2 x_i, -|x_i|^2, -1), v_j = (x_j, 1, |x_j|^2) the PE
   matmul u.v directly yields -D, and one activation computes
   w = Exp((-D) * 1/gamma) from PSUM.  Only a |j-i|<16 band is needed:
   the SoftDTW Gibbs weights decay like exp(-|D|/gamma) ~ 3e-6 per
   off-diagonal step, so the band truncation error is ~e^-200.
2. Band gather: the [i-part, j-free] tiles round-trip through a DRAM
   scratch buffer; the re-read uses a diagonal (stride 161) access
   pattern, landing W in scan-ready [sig-part, (row, delta)] layout.
3. DP (Vector engine): in probability domain P = exp(-R/gamma) the
   SoftDTW recurrence is linear:  P[i,j] = w*(P[i-1,j-1] + P[i-1,j]
   + P[i,j-1]), i.e. per row one pair-sum (tensor_tensor add) and one
   hardware scan (tensor_tensor_scan, state=(up+state)*w). 512 serial
   rows; drains guard the same-engine RAW pipeline hazard.
4. Diagonal extract + host epilogue: R(L) = -gamma*ln(P[L-1,L-1]),
   dists = R/(2L), then the tiny triplet-margin reduction in numpy.
"""
import numpy as np

import concourse.bass as bass
import concourse.mybir as mybir
from concourse.bass_utils import run_bass_kernel_spmd

NG_, NF_, NW_ = 5, 10, 8
STEP = NG_ + NF_ + 1            # 16 signatures per writer
MARGIN = np.float32(1.0)
MODEL_LAMBDA = np.float32(0.01)
GAMMA = np.float32(5.0)

B, N, F = 128, 512, 32
NCORES = 8
S = B // NCORES                 # 16 signatures per core
HB = 16                         # half band width
BW = 2 * HB                     # 32 band slots, delta = j - i + HB
SW = BW + 1                     # stored row width (slot BW is a zero pad)
NB = N // 128                   # 4 row blocks of 128
TW = 128 + BW                   # 160 j-columns produced per row block
K = F + 2                       # augmented feature dim
VW = N + BW                     # 544 padded V columns per signature
PADC = np.float32(50.0)         # pad column makes -D ~ -50*(|x|^2+1) -> w=0


def _build_core_kernel():
    nc = bass.Bass()
    vx = nc.declare_dram_parameter("vx", [F, S * N], mybir.dt.float8e4, isOutput=False)
    sdiag = nc.declare_dram_parameter("sdiag", [S, N], mybir.dt.float32, isOutput=True)
    wdd = nc.dram_tensor("wdd", [S * NB * 128 * TW], mybir.dt.bfloat16)
    # NEFF-embedded constants (no per-call upload): VAsq row0 = ones/PADC
    # complete; row1 = PADC (sq lands later via DMA).  UAsq row1 = -1.
    cva = np.full((2, S * VW), PADC, np.float32)
    ones_pads = np.full((S, VW), PADC, np.float32)
    ones_pads[:, HB:HB + N] = 1.0
    cva[0] = ones_pads.reshape(-1)
    c_vasq = nc.inline_tensor(cva, "c_vasq")
    cua = np.zeros((2, S * N), np.float32)
    cua[1] = -0.5
    c_uasq = nc.inline_tensor(cua, "c_uasq")

    from contextlib import ExitStack
    with ExitStack() as es:
        UAsq = es.enter_context(nc.sbuf_tensor([2, S * N], mybir.dt.float32))
        VAx = es.enter_context(nc.sbuf_tensor([F, S * VW], mybir.dt.float8e4))
        VAsq = es.enter_context(nc.sbuf_tensor([2, S * VW], mybir.dt.float32))
        WT0 = es.enter_context(nc.sbuf_tensor([128, TW], mybir.dt.bfloat16))
        WT1 = es.enter_context(nc.sbuf_tensor([128, TW], mybir.dt.bfloat16))
        WB = es.enter_context(nc.sbuf_tensor([S, N * BW], mybir.dt.bfloat16))
        SR = es.enter_context(nc.sbuf_tensor([S, N * SW], mybir.dt.float32))
        UP = es.enter_context(nc.sbuf_tensor([S, BW], mybir.dt.float32))
        UP0 = es.enter_context(nc.sbuf_tensor([S, BW], mybir.dt.float32))
        SD = es.enter_context(nc.sbuf_tensor([S, N], mybir.dt.float32))
        XQ0 = es.enter_context(nc.sbuf_tensor([F, N], mybir.dt.float32))
        XQ1 = es.enter_context(nc.sbuf_tensor([F, N], mybir.dt.float32))
        STG = es.enter_context(nc.sbuf_tensor([1, N], mybir.dt.float32))
        ONE = es.enter_context(nc.sbuf_tensor([F, 1], mybir.dt.float32))
        P0 = es.enter_context(nc.psum_tensor([128, TW], mybir.dt.float32))
        P1 = es.enter_context(nc.psum_tensor([128, TW], mybir.dt.float32))
        PQ = es.enter_context(nc.psum_tensor([1, N], mybir.dt.float32))
        s_in = es.enter_context(nc.semaphore("s_in"))
        s_ms = es.enter_context(nc.semaphore("s_ms"))
        s_xsq = es.enter_context(nc.semaphore("s_xsq"))
        s_mmq = es.enter_context(nc.semaphore("s_mmq"))
        s_sq = es.enter_context(nc.semaphore("s_sq"))
        s_sqd = es.enter_context(nc.semaphore("s_sqd"))
        s_prep = es.enter_context(nc.semaphore("s_prep"))
        s_mm = es.enter_context(nc.semaphore("s_mm"))
        s_exp = es.enter_context(nc.semaphore("s_exp"))
        s_wr = es.enter_context(nc.semaphore("s_wr"))
        s_band = es.enter_context(nc.semaphore("s_band"))
        s_dp = es.enter_context(nc.semaphore("s_dp"))
        s_ext = es.enter_context(nc.semaphore("s_ext"))
        s_out = es.enter_context(nc.semaphore("s_out"))
        block = es.enter_context(nc.Block())
        WT = (WT0, WT1)
        PP = (P0, P1)

        @block.sync
        def _(sync):
            # unpadded x lands in the non-pad columns of VAx
            sync.dma_start(
                out=bass.AP(VAx, HB, [[S * VW, F], [VW, S], [1, N]]),
                in_=bass.AP(vx, 0, [[S * N, F], [N, S], [1, N]]),
            ).then_inc(s_in, 16)
            sync.dma_start(out=VAsq[:, :], in_=c_vasq[:, :]).then_inc(s_in, 16)
            sync.dma_start(out=UAsq[:, :], in_=c_uasq[:, :]).then_inc(s_in, 16)
            # device-computed sq rows -> VAsq row 1, one sig at a time
            # (engines cannot write partition 1; SBUF->SBUF DMAs can)
            for s in range(S):
                sync.wait_ge(s_sq, 2 * s + 1)
                sync.dma_start(
                    out=VAsq[1:2, s * VW + HB: s * VW + HB + N],
                    in_=STG[:, :],
                ).then_inc(s_sqd, 16)
            for ib in range(NB):
                for s in range(S):
                    k = ib * S + s
                    sync.wait_ge(s_exp, k + 1)
                    # plain contiguous write of the [128, TW] tile
                    sync.dma_start(
                        out=bass.AP(wdd, (s * NB + ib) * 128 * TW,
                                    [[TW, 128], [1, TW]]),
                        in_=WT[k % 2][:, :],
                    ).then_inc(s_wr, 16)
                sync.wait_ge(s_wr, 16 * S * (ib + 1))
                # diagonal band re-read: for (s, p, d):
                #   src elem = (s*NB+ib)*128*TW + p*(TW+1) + d
                sync.dma_start(
                    out=bass.AP(WB, ib * 128 * BW,
                                [[N * BW, S], [BW, 128], [1, BW]]),
                    in_=bass.AP(wdd, ib * 128 * TW,
                                [[NB * 128 * TW, S], [TW + 1, 128], [1, BW]]),
                ).then_inc(s_band, 16)
            sync.wait_ge(s_ext, 1)
            sync.dma_start(out=sdiag[:, :], in_=SD[:, :]).then_inc(s_out, 16)
            sync.wait_ge(s_out, 16)

        @block.tensor
        def _(tensor):
            tensor.wait_ge(s_ms, 3)
            # per-sig sq reduction: PQ[0, :] = sum_d XQ[d, :]
            for s in range(S):
                tensor.wait_ge(s_xsq, s + 1)
                if s >= 1:
                    tensor.wait_ge(s_sq, 2 * s)
                tensor.matmul(PQ[:, :], ONE[:, :], (XQ0 if s % 2 == 0 else XQ1)[:, :],
                              start=True, stop=True).then_inc(s_mmq, 1)
            tensor.wait_ge(s_sqd, 16 * S)
            for ib in range(NB):
                for s in range(S):
                    k = ib * S + s
                    if k >= 2:
                        tensor.wait_ge(s_exp, k - 1)
                    tensor.matmul(
                        PP[k % 2][:, :],
                        VAx[:, s * VW + HB + ib * 128:
                            s * VW + HB + ib * 128 + 128],
                        VAx[:, s * VW + ib * 128: s * VW + ib * 128 + TW],
                        start=True, stop=False,
                    )
                    tensor.matmul(
                        PP[k % 2][:, :],
                        UAsq[:, s * N + ib * 128: s * N + ib * 128 + 128],
                        VAsq[:, s * VW + ib * 128: s * VW + ib * 128 + TW],
                        start=False, stop=True,
                    ).then_inc(s_mm, 1)

        @block.scalar
        def _(scalar):
            # Wait for ALL input DMAs: completions may land out of issue
            # order across queues.
            scalar.wait_ge(s_in, 48)
            # per-sig: square x (f32), then stage sq and -sq/2 rows
            for s in range(S):
                if s >= 2:
                    scalar.wait_ge(s_mmq, s - 1)
                scalar.activation(
                    (XQ0 if s % 2 == 0 else XQ1)[:, :],
                    VAx[0:F, s * VW + HB: s * VW + HB + N],
                    mybir.ActivationFunctionType.Square,
                ).then_inc(s_xsq, 1)
                scalar.wait_ge(s_mmq, s + 1)
                if s >= 1:
                    scalar.wait_ge(s_sqd, 16 * s)
                scalar.copy(STG[:, :], PQ[:, :]).then_inc(s_sq, 1)
                scalar.mul(UAsq[0:1, s * N:(s + 1) * N], PQ[:, :], -0.5
                           ).then_inc(s_sq, 1)
            for k in range(NB * S):
                scalar.wait_ge(s_mm, k + 1)
                if k >= 2:
                    scalar.wait_ge(s_wr, 16 * (k - 1))
                scalar.activation(
                    WT[k % 2][:, :], PP[k % 2][:, :],
                    mybir.ActivationFunctionType.Exp,
                    bias=0.0, scale=float(2.0 / GAMMA),
                ).then_inc(s_exp, 1)
            scalar.wait_ge(s_dp, 1)
            scalar.copy(SD[:, :], bass.AP(SR, HB, [[N * SW, S], [SW, N]])
                        ).then_inc(s_ext, 1)

        @block.vector
        def _(vector):
            vector.memset(bass.AP(VAx, 0, [[S * VW, F], [VW, S], [1, HB]]), 0.0
                          ).then_inc(s_ms, 1)
            vector.memset(bass.AP(VAx, HB + N, [[S * VW, F], [VW, S], [1, HB]]), 0.0
                          ).then_inc(s_ms, 1)
            vector.memset(ONE[:, :], 1.0).then_inc(s_ms, 1)
            vector.memset(SR[:, :], 0.0)
            vector.memset(UP0[:, :], 0.0)
            vector.memset(UP0[:, HB:HB + 1], 1.0)
            vector.drain()
            for ib in range(NB):
                vector.wait_ge(s_band, 16 * (ib + 1))
                for i in range(ib * 128, ib * 128 + 128):
                    if i == 0:
                        vector.tensor_tensor_scan(
                            SR[:, 0:BW], UP0[:, :], WB[:, 0:BW], 0.0,
                            mybir.AluOpType.add, mybir.AluOpType.mult)
                        continue
                    po = (i - 1) * SW
                    vector.drain()
                    vector.tensor_tensor(
                        UP[:, :], SR[:, po:po + BW], SR[:, po + 1:po + BW + 1],
                        mybir.AluOpType.add)
                    vector.drain()
                    vector.tensor_tensor_scan(
                        SR[:, i * SW:i * SW + BW], UP[:, :],
                        WB[:, i * BW:(i + 1) * BW], 0.0,
                        mybir.AluOpType.add, mybir.AluOpType.mult)
            vector.engine_nop().then_inc(s_dp, 1)

    return nc


_NC = None


def _get_nc():
    global _NC
    if _NC is None:
        _NC = _build_core_kernel()
    return _NC


def _prep_inputs(data):
    """Only x ships (unpadded, fp8 e4m3: validated end-to-end loss shift
    ~1e-7); the device derives everything else.  sq is computed on device
    FROM the fp8 values (in f32) so the matmul's D[i,i] cancels exactly."""
    fp8 = mybir.dt.np(mybir.dt.float8e4)
    x = data.reshape(NCORES, S, N, F)
    xT = x.astype(fp8).transpose(0, 3, 1, 2)                      # [C, F, S, N]
    return np.ascontiguousarray(xT.reshape(NCORES, F, S * N))


_DISPATCH = None


def _get_dispatch():
    """Build the sharded jitted executable ONCE (run_bass_via_pjrt re-traces
    per call; this mirrors its multi-core path with a cached jit)."""
    global _DISPATCH
    if _DISPATCH is None:
        import jax
        from jax.sharding import Mesh, PartitionSpec
        from jax.experimental.shard_map import shard_map
        from concourse import bass2jax as b2j

        nc = _get_nc()
        b2j.install_neuronx_cc_hook()
        partition_name = (nc.partition_id_tensor.name
                          if nc.partition_id_tensor else None)
        in_names, out_names, out_avals = [], [], []
        out_shapes = []
        for alloc in nc.m.functions[0].allocations:
            if not isinstance(alloc, mybir.MemoryLocationSet):
                continue
            name = alloc.memorylocations[0].name
            if alloc.kind == "ExternalInput":
                if name != partition_name:
                    in_names.append(name)
            elif alloc.kind == "ExternalOutput":
                out_names.append(name)
                shape = tuple(alloc.tensor_shape)
                dtype = mybir.dt.np(alloc.dtype)
                out_avals.append(jax.core.ShapedArray(shape, dtype))
                out_shapes.append((shape, dtype))
        n_params = len(in_names)
        n_outs = len(out_names)
        bind_names = list(in_names) + list(out_names)
        if partition_name is not None:
            bind_names.append(partition_name)
        donate = tuple(range(n_params, n_params + n_outs))

        def _body(*args):
            operands = list(args)
            if partition_name is not None:
                operands.append(b2j.partition_id_tensor())
            outs = b2j._bass_exec_p.bind(
                *operands,
                out_avals=tuple(out_avals),
                in_names=tuple(bind_names),
                out_names=tuple(out_names),
                lowering_input_output_aliases=(),
                sim_require_finite=True,
                sim_require_nnan=True,
                nc=nc,
            )
            return tuple(outs)

        devices = jax.devices()[:NCORES]
        mesh = Mesh(np.asarray(devices), ("core",))
        in_specs = (PartitionSpec("core"),) * (n_params + n_outs)
        out_specs = (PartitionSpec("core"),) * n_outs
        sharded = jax.jit(
            shard_map(_body, mesh=mesh, in_specs=in_specs,
                      out_specs=out_specs, check_rep=False),
            donate_argnums=donate, keep_unused=True)
        _DISPATCH = (sharded, list(in_names), out_shapes)
    return _DISPATCH


_FAST_OK = True


def _run_device(vax):
    """Fast path: cached jitted shard_map dispatch.  Falls back to the
    stock run_bass_kernel_spmd if the cached-jit internals ever break."""
    global _FAST_OK
    if _FAST_OK:
        try:
            sharded, in_names, out_shapes = _get_dispatch()
            assert in_names == ["vx"], in_names
            concat_in = [np.ascontiguousarray(vax.reshape(NCORES * F, S * N))]
            concat_zeros = [np.zeros((NCORES * shp[0],) + shp[1:], dt)
                            for shp, dt in out_shapes]
            out_arrs = sharded(*concat_in, *concat_zeros)
            return np.asarray(out_arrs[0]).reshape(B, N)
        except Exception:
            _FAST_OK = False
    nc = _get_nc()
    in_maps = [{"vx": np.ascontiguousarray(vax[c])} for c in range(NCORES)]
    res = run_bass_kernel_spmd(nc, in_maps, list(range(NCORES)))
    return np.concatenate([res.results[c]["sdiag"] for c in range(NCORES)], 0)


def kernel(data: np.ndarray, lens: np.ndarray) -> np.ndarray:
    data = np.asarray(data, np.float32)
    lens = np.asarray(lens, np.int32)

    vax = _prep_inputs(data)
    sdiag = _run_device(vax)

    L = np.clip(lens, 1, N).astype(np.int64)
    pll = sdiag[np.arange(B), L - 1]
    R = (-GAMMA * np.log(np.maximum(pll, np.float32(1e-30)))).astype(np.float32)
    dists = (R / (np.float32(2.0) * L.astype(np.float32))).astype(np.float32)

    d = dists.reshape(NW_, STEP)
    dm = ((d[:, :, None] + d[:, None, :]) * np.float32(0.5)).astype(np.float32)
    g = NG_ + 1
    dmg = dm[:, :g, :g]
    neg = dm[:, :g, g:]
    scores = np.maximum(dmg[:, :, :, None] + MARGIN - neg[:, :, None, :],
                        np.float32(0.0))
    maxj = scores.max(axis=(2, 3)).astype(np.float32)
    sum_lks = maxj.sum(axis=1) * np.float32(g * NF_)
    nnz = (maxj != 0).astype(np.float32).sum(axis=1) * np.float32(g * NF_)
    lv = sum_lks / (nnz + np.float32(1.0))
    tril = np.tril(np.ones((g, g), bool), k=-1)
    only_pos = np.where(tril[None], dmg, np.float32(0.0)).sum(axis=(1, 2)) * (
        MODEL_LAMBDA / np.float32(NG_))
    loss = (lv + only_pos).sum() / np.float32(NW_)
    return np.float32(loss)
